# revision 20
# baseline (speedup 1.0000x reference)
"""Trainium2 Bass kernel for BottleneckAttention (patch attention).

q patches [160, 5120] from z1_hat (non-overlapping 10x4 unfold),
kv patches [5551, 5120] from z2 (overlapping unfold, Hk=91 x Wk=61),
scores = q @ kv.T / 5120, softmax over kv patches, out = attn @ kv,
folded back to [1, 128, 100, 64].

Sharding: 12 kv h-rows (768 flat positions) per core; every core computes
all 160 q columns; host combines with an all-gather softmax using the
centered form f = e - 1 (the exact colsum term is added in fp64 on host).

Per-core kernel (v17): every matmul is an fp8e4 DoubleRow matmul (two
128-contraction k-tiles per ~78 ns instruction), and phase 1 layers a
Winograd F(2,2) scheme over the 4 w-taps on top of that:
  phase 1 computes scores TRANSPOSED [pos, q] in PARITY-SPLIT order
    (chunk ci = 2*blk + par holds positions x = 2*(128*blk + p) + par).
    Per position-pair block (3 blocks of 128 pairs), three m-chains
      m1 = d1*(g0+g1), m2 = (d0-d1)*g0, m3 = (d1-d2)*(-g1)
    accumulate over 10 DoubleRow each (i-pairs (2pi, 2pi+1) as k-tiles,
    stride-32 overlapping windows on host de-interleaved slabs), so the
    240 tap-matmuls become 90 DR. Sweep 1 runs blocks 0/1 (q-piece
    paced); sweep 2 runs block 2 with its m3 chain deferred until
    VectorE frees bank 0. VectorE forms s_even = m1+m2 / s_odd = m1+m3
    (m1 staged via SBUF), ScalarE applies exp, VectorE applies
    f = (e-1)*mask in fp8.
  phase 2 computes out TRANSPOSED [(c,i,j), q]: per tap, 3 DoubleRow
    whose k-tiles are the parity pair of each position-pair block:
    stationary = consecutive slots of the host ztp (parity/shift
    de-interleaved z2T windows, slot = 6*rank(4B+i) + j, slots ordered
    by tap-batch need so the three zt DMA pieces gate batches 0/1/2),
    moving = f chunk pairs. Batch 0 (taps 0-7) runs B-major so the
    phase-1 drain tail hides behind its B=0/B=1 passes. PSUM->SBUF
    copies alternate ScalarE/VectorE; fp16 output.
  denominator = ones-vector matmul chain after tap 28 in bank 2, shipped
    immediately on the Activation queue.
Input DMAs ride both HW-DGE queues (Activation: qw pieces 0/2/4 + ztp
pieces; SP/Sync: zw3, qw 1/3, mask) in consumption order -- DMA
completion has a ~2 us notify latency, so phase 1's start is
input-bound; warmup matmuls keep the PE p-state ramped until the first
q piece lands. Outputs ride the SP queue in 8-tap groups as copies
complete, with the last group split off on the Activation queue.
"""

import sys

sys.path.insert(0, "/opt/trn_rl_repo")

import numpy as np
import ml_dtypes

import concourse.bass as bass
import concourse.mybir as mybir
import bass_rust

DT = mybir.dt
AF = mybir.ActivationFunctionType
ALU = mybir.AluOpType
PM = mybir.MatmulPerfMode

# problem geometry (hardcoded from the reference module)
KC, KH, KW = 128, 10, 4
H, W = 100, 64
NH, NW = H // KH, W // KW          # 10, 16
PQ = NH * NW                       # 160 q patches
D = KC * KH * KW                   # 5120
HK, WK = H - KH + 1, W - KW + 1    # 91, 61
NCORES = 8
HPC = 12                           # kv h-rows per core
NPOS = 24 * W                      # 1536 slab positions per core
NOWN = HPC * W                     # 768 owned positions per core
NB = NOWN // 128                   # 6 score/f chunks of 128 positions
NU = NOWN                          # 768 de-interleaved slab length
NSP = 10                           # phase-1 step-pairs (5 i-pairs x 2 jp)
NIJ = KH * KW                      # 40 (i,j) output taps
NSLOT = 108                        # ztp slots (18 k-groups x 6)
SCALE = 1.0 / D

F8 = ml_dtypes.float8_e4m3

_CACHE = {}

# ztp k-groups (k = 4*B + i) ordered by tap-batch need: batch0 uses
# k {0,1,4,5,8,9}, batch1 {2,3,6,7,10,11}, batch2+ the rest
KORDER = (0, 1, 4, 5, 8, 9, 2, 3, 6, 7, 10, 11, 12, 13, 14, 15, 16, 17)
KRANK = tuple(KORDER.index(k) for k in range(18))

# phase-1 m-chain banks per pair-block
BANKS = {0: (0, 1, 2), 1: (3, 4, 5), 2: (6, 7, 0)}
# phase-2 tap -> psum bank, in phase-1 drain order
PERM = (1, 2, 3, 4, 5, 6, 7, 0)


def _build_nc():
    nc = bass.Bass()
    zw_d = nc.declare_dram_parameter("zw3", [KC, 3, NU], DT.float8e4, isOutput=False)
    q_d = nc.declare_dram_parameter(
        "qw", [KC, NSP, 6 * PQ], DT.float8e4, isOutput=False
    )
    zt_d = nc.declare_dram_parameter(
        "ztp", [128, NSLOT, KC], DT.float8e4, isOutput=False
    )
    mk_d = nc.declare_dram_parameter("msk", [128, NB], DT.float32, isOutput=False)
    out_d = nc.declare_dram_parameter("out", [KC, NIJ, PQ], DT.float16, isOutput=True)
    den_d = nc.declare_dram_parameter("den", [1, PQ], DT.float32, isOutput=True)

    from contextlib import ExitStack

    ctx = ExitStack()
    with ctx:
        zw_sb = ctx.enter_context(nc.sbuf_tensor([KC, 3, NU], DT.float8e4))
        q_sb = ctx.enter_context(nc.sbuf_tensor([KC, NSP, 6 * PQ], DT.float8e4))
        zt_sb = ctx.enter_context(nc.sbuf_tensor([128, NSLOT, KC], DT.float8e4))
        mk_sb = ctx.enter_context(nc.sbuf_tensor([128, NB], DT.float32))
        m1_sb = ctx.enter_context(nc.sbuf_tensor([128, 3, PQ], DT.float32))
        se_sb = ctx.enter_context(nc.sbuf_tensor([128, NB, PQ], DT.float32))
        e_sb = ctx.enter_context(nc.sbuf_tensor([128, NB, PQ], DT.float32))
        f_sb = ctx.enter_context(nc.sbuf_tensor([128, NB, PQ], DT.float8e4))
        o_sb = ctx.enter_context(nc.sbuf_tensor([128, NIJ, PQ], DT.float16))
        den_sb = ctx.enter_context(nc.sbuf_tensor([1, PQ], DT.float32))
        ones_sb = ctx.enter_context(nc.sbuf_tensor([128, 1], DT.float8e4))
        wz = ctx.enter_context(nc.sbuf_tensor([128, 128], DT.float8e4))

        ps = [
            ctx.enter_context(nc.psum_tensor(f"ps{i}", [128, 512], DT.float32))
            for i in range(8)
        ]

        s_wz = ctx.enter_context(nc.semaphore("s_wz"))
        s_izw = ctx.enter_context(nc.semaphore("s_izw"))
        s_qs = [ctx.enter_context(nc.semaphore(f"s_q{i}")) for i in range(5)]
        s_im = ctx.enter_context(nc.semaphore("s_im"))
        s_izt = [ctx.enter_context(nc.semaphore(f"s_izt{i}")) for i in range(3)]
        s_p = ctx.enter_context(nc.semaphore("s_p"))
        s_m1 = ctx.enter_context(nc.semaphore("s_m1"))
        s_add = ctx.enter_context(nc.semaphore("s_add"))
        s_exp = ctx.enter_context(nc.semaphore("s_exp"))
        s_f = ctx.enter_context(nc.semaphore("s_f"))
        s_cpa = ctx.enter_context(nc.semaphore("s_cpa"))
        s_cpv = ctx.enter_context(nc.semaphore("s_cpv"))
        s_den = ctx.enter_context(nc.semaphore("s_den"))
        s_o = ctx.enter_context(nc.semaphore("s_o"))

        # s_p: sweep1 chains 1..6 (blk0 c012, blk1 c012), sweep2 7..9;
        #      taps 0..28 -> 10..38; den -> 39; taps 29..39 -> 40..50
        DEN_AT = 28

        def sp_tap(g):
            return 10 + g if g <= DEN_AT else 11 + g

        def zw_pair(blk, u, c):
            # overlapping [128, 2, 128] windows (stride 32) on comp slab c:
            # k-tiles are steps (i=2pi, 2pi+1) for u = 2pi + jp
            off = 128 * blk + 64 * (u // 2) + (u % 2)
            ap = zw_sb[:, c, off : off + 128].copy()
            ap.ap = bass_rust.VecI64Pair([[3 * NU, 128], [32, 2], [1, 128]])
            return ap

        def q_pair(u, c):
            # contiguous comp pair [128, 2, 160] inside the flat q piece
            ap = q_sb[:, u, 2 * PQ * c : 2 * PQ * (c + 1)].copy()
            ap.ap = bass_rust.VecI64Pair([[NSP * 6 * PQ, 128], [PQ, 2], [1, PQ]])
            return ap

        with nc.Block() as block:

            @block.sync
            def _(sync):
                # head inputs interleaved with the Activation queue
                sync.dma_start(zw_sb[:], zw_d[:]).then_inc(s_izw, 16)
                sync.dma_start(q_sb[:, 2:4], q_d[:, 2:4]).then_inc(s_qs[1], 16)
                sync.dma_start(q_sb[:, 6:8], q_d[:, 6:8]).then_inc(s_qs[3], 16)
                sync.dma_start(mk_sb[:], mk_d[:]).then_inc(s_im, 16)
                for b in range(4):
                    sl = slice(8 * b, 8 * b + 8)
                    sync.wait_ge(s_cpa, 4 * (b + 1))
                    sync.wait_ge(s_cpv, 4 * (b + 1))
                    sync.dma_start(out_d[:, sl, :], o_sb[:, sl, :]).then_inc(
                        s_o, 16
                    )
                sync.wait_ge(s_cpa, 18)
                sync.wait_ge(s_cpv, 18)
                sync.dma_start(out_d[:, 32:36, :], o_sb[:, 32:36, :]).then_inc(
                    s_o, 16
                )
                sync.wait_ge(s_cpa, 19)
                sync.wait_ge(s_cpv, 19)
                sync.dma_start(out_d[:, 36:38, :], o_sb[:, 36:38, :]).then_inc(
                    s_o, 16
                )
                sync.wait_ge(s_o, 128)

            @block.tensor
            def _(pe):
                # warmup on the zeroed wz tile while input DMAs land; also
                # ramps the PE p-state so phase 1 runs near full clock
                pe.wait_ge(s_wz, 1)
                for w_ in range(27):
                    nc.tensor.matmul(
                        ps[7][0:128, 0:128],
                        wz[:, 0:128],
                        wz[:, 0:128],
                        start=(w_ == 0),
                        stop=(w_ == 26),
                    )
                pe.wait_ge(s_izw, 16)

                def p1_mm(blk, u, c, start, stop):
                    return nc.tensor.matmul(
                        ps[BANKS[blk][c]][0:128, 0:PQ],
                        zw_pair(blk, u, c),
                        q_pair(u, c),
                        start=start,
                        stop=stop,
                        perf_mode=PM.DoubleRow,
                    )

                # sweep 1: blocks 0,1 (6 m-chains, banks 0..5), q-paced
                for u in range(NSP):
                    if u % 2 == 0:
                        pe.wait_ge(s_qs[u // 2], 16)
                    for blk in (0, 1):
                        for c in range(3):
                            mm = p1_mm(blk, u, c, u == 0, u == NSP - 1)
                            if u == NSP - 1:
                                mm.then_inc(s_p, 1)  # 1..6
                # sweep 2: block 2 (banks 6,7 + deferred m3 in bank 0)
                for u in range(NSP):
                    if u == 4:
                        pe.wait_ge(s_m1, 1)  # bank 0 freed (B0 m1 staged)
                    mm = p1_mm(2, u, 0, u == 0, u == NSP - 1)
                    if u == NSP - 1:
                        mm.then_inc(s_p, 1)  # 7
                    mm = p1_mm(2, u, 1, u == 0, u == NSP - 1)
                    if u == NSP - 1:
                        mm.then_inc(s_p, 1)  # 8
                    if u >= 4:
                        p1_mm(2, u - 4, 2, u == 4, False)
                for u2 in range(NSP - 4, NSP):
                    mm = p1_mm(2, u2, 2, False, u2 == NSP - 1)
                mm.then_inc(s_p, 1)  # 9

                # phase 2: tap g -> bank PERM[g%8], 3 DoubleRow per tap
                def p2_mm(g, B, start, stop):
                    i_, j_ = g // KW, g % KW
                    slot = 6 * KRANK[4 * B + i_] + j_
                    return nc.tensor.matmul(
                        ps[PERM[g % 8]][0:128, 0:PQ],
                        zt_sb[:, slot : slot + 2, :],
                        f_sb[:, 2 * B : 2 * B + 2, :],
                        start=start,
                        stop=stop,
                        perf_mode=PM.DoubleRow,
                    )

                # batch 0 B-major: start as soon as f0/f1 exist; per-tap
                # gates release banks as the phase-1 drain proceeds
                pe.wait_ge(s_izt[0], 16)
                pe.wait_ge(s_f, 2)
                B0GATE = {
                    2: (s_m1, 2),
                    3: (s_add, 3),
                    4: (s_add, 4),
                    5: (s_m1, 3),
                    6: (s_add, 5),
                    7: (s_add, 6),
                }
                for g in range(8):
                    if g in B0GATE:
                        pe.wait_ge(*B0GATE[g])
                    p2_mm(g, 0, True, False)
                pe.wait_ge(s_f, 4)
                for g in range(8):
                    p2_mm(g, 1, False, False)
                pe.wait_ge(s_f, NB)
                for g in range(8):
                    p2_mm(g, 2, False, True).then_inc(s_p, 1)
                # batches 1..4 g-major
                for g in range(8, NIJ):
                    gp = g - 8
                    if g == 8:
                        pe.wait_ge(s_izt[1], 16)
                    elif g == 16:
                        pe.wait_ge(s_izt[2], 16)
                    if g % 2 == 0:
                        pe.wait_ge(s_cpa, gp // 2 + 1)
                    else:
                        pe.wait_ge(s_cpv, gp // 2 + 1)
                    if g == 33:
                        # bank 2 was reused by the den chain: wait its copy
                        pe.wait_ge(s_den, 1)
                    p2_mm(g, 0, True, False)
                    p2_mm(g, 1, False, False)
                    p2_mm(g, 2, False, True).then_inc(s_p, 1)
                    if g == DEN_AT:
                        # denominator: ones.T @ f -> [1, 160] in bank 2
                        # (tap 25's copy freed it: s_cpv >= 13)
                        pe.wait_ge(s_wz, 2)
                        pe.wait_ge(s_cpv, 13)
                        for ci in range(NB):
                            mm = nc.tensor.matmul(
                                ps[2][0:1, 0:PQ],
                                ones_sb[0:128, 0:1],
                                f_sb[:, ci, :],
                                start=(ci == 0),
                                stop=(ci == NB - 1),
                            )
                        mm.then_inc(s_p, 1)  # s_p = 39

            @block.scalar
            def _(act):
                # input DMAs on the Activation HW-DGE queue, consumption order
                act.dma_start(q_sb[:, 0:2], q_d[:, 0:2]).then_inc(s_qs[0], 16)
                act.dma_start(q_sb[:, 4:6], q_d[:, 4:6]).then_inc(s_qs[2], 16)
                act.dma_start(q_sb[:, 8:10], q_d[:, 8:10]).then_inc(s_qs[4], 16)
                act.dma_start(zt_sb[:, 0:36, :], zt_d[:, 0:36, :]).then_inc(
                    s_izt[0], 16
                )
                act.dma_start(zt_sb[:, 36:72, :], zt_d[:, 36:72, :]).then_inc(
                    s_izt[1], 16
                )
                act.dma_start(zt_sb[:, 72:NSLOT, :], zt_d[:, 72:NSLOT, :]).then_inc(
                    s_izt[2], 16
                )
                for ci in range(NB):
                    act.wait_ge(s_add, ci + 1)
                    nc.scalar.activation(
                        e_sb[:, ci, :], se_sb[:, ci, :], AF.Exp, scale=SCALE
                    ).then_inc(s_exp, 1)
                for g in range(0, NIJ, 2):
                    act.wait_ge(s_p, sp_tap(g))
                    nc.scalar.activation(
                        o_sb[:, g, :], ps[PERM[g % 8]][0:128, 0:PQ], AF.Copy
                    ).then_inc(s_cpa, 1)
                    if g == DEN_AT:
                        act.wait_ge(s_p, 39)
                        nc.scalar.activation(
                            den_sb[0:1, 0:PQ], ps[2][0:1, 0:PQ], AF.Copy
                        ).then_inc(s_den, 1)
                        act.wait_ge(s_den, 1)
                        act.dma_start(
                            den_d[:, :], den_sb[0:1, 0:PQ]
                        ).then_inc(s_o, 16)
                # tail outputs on this queue so they overlap the SP groups
                act.wait_ge(s_cpa, 20)
                act.wait_ge(s_cpv, 20)
                act.dma_start(out_d[:, 38:NIJ, :], o_sb[:, 38:NIJ, :]).then_inc(
                    s_o, 16
                )

            @block.vector
            def _(dve):
                nc.vector.memset(wz[:], 0.0).then_inc(s_wz, 1)
                nc.vector.memset(ones_sb[:], 1.0).then_inc(s_wz, 1)
                dve.wait_ge(s_im, 16)  # mask resident

                def psum_sum(blk, par, ci):
                    # s_even = m1+m2; s_odd = m1+m3 (m1 staged via SBUF:
                    # the DVE may read at most one PSUM operand)
                    b1, b2, b3 = BANKS[blk]
                    if par == 0:
                        nc.vector.tensor_copy(
                            m1_sb[:, blk, :], ps[b1][0:128, 0:PQ]
                        ).then_inc(s_m1, 1)
                    dve.wait_ge(s_m1, blk + 1)
                    other = b2 if par == 0 else b3
                    nc.vector.tensor_tensor(
                        se_sb[:, ci, :],
                        m1_sb[:, blk, :],
                        ps[other][0:128, 0:PQ],
                        ALU.add,
                    ).then_inc(s_add, 1)

                def fop(ci):
                    dve.wait_ge(s_exp, ci + 1)
                    nc.vector.tensor_scalar(
                        f_sb[:, ci, :],
                        e_sb[:, ci, :],
                        -1.0,
                        mk_sb[:, ci : ci + 1],
                        ALU.add,
                        ALU.mult,
                    ).then_inc(s_f, 1)

                dve.wait_ge(s_p, 3)
                psum_sum(0, 0, 0)
                psum_sum(0, 1, 1)
                dve.wait_ge(s_p, 6)
                psum_sum(1, 0, 2)
                psum_sum(1, 1, 3)
                fop(0)
                fop(1)
                fop(2)
                fop(3)
                dve.wait_ge(s_p, 9)
                psum_sum(2, 0, 4)
                psum_sum(2, 1, 5)
                fop(4)
                fop(5)
                for g in range(1, NIJ, 2):
                    dve.wait_ge(s_p, sp_tap(g))
                    nc.vector.tensor_copy(
                        o_sb[:, g, :], ps[PERM[g % 8]][0:128, 0:PQ]
                    ).then_inc(s_cpv, 1)

    return nc


def _host_prep(z1_hat, z2):
    z1 = np.asarray(z1_hat, dtype=np.float32)[0]   # [128, 100, 64]
    z2a = np.asarray(z2, dtype=np.float32)[0]

    # q winograd transform with DoubleRow i-pair interleave:
    # qw[c, u=2pi+jp, comp, t, pq] for tap i = 2pi+t, w-taps (2jp, 2jp+1)
    q = z1.reshape(KC, NH, KH, NW, KW).transpose(1, 3, 0, 2, 4).reshape(PQ, D)
    q4 = q.reshape(PQ, KC, KH, KW).transpose(1, 2, 3, 0)   # [128, 10, 4, 160]
    qw = np.zeros((KC, NSP, 3, 2, PQ), dtype=np.float32)
    for pi in range(5):
        for jp in range(2):
            u = 2 * pi + jp
            for t in range(2):
                i = 2 * pi + t
                g0, g1 = q4[:, i, 2 * jp], q4[:, i, 2 * jp + 1]
                qw[:, u, 0, t] = g0 + g1
                qw[:, u, 1, t] = g0
                qw[:, u, 2, t] = -g1
    qw = np.ascontiguousarray(qw.reshape(KC, NSP, 6 * PQ).astype(F8))

    z_pad = np.zeros((KC, 112, W), dtype=np.float32)
    z_pad[:, :H] = z2a

    in_maps = []
    p = np.arange(128)
    for core in range(NCORES):
        h0 = HPC * core
        slab = z_pad[:, h0 : h0 + 24, :].reshape(KC, NPOS)  # [128, 1536] f32
        zd = np.zeros((KC, NPOS), dtype=np.float32)
        zd[:, : NPOS - 1] = slab[:, : NPOS - 1] - slab[:, 1:]
        zw3 = np.zeros((KC, 3, NU), dtype=np.float32)
        zw3[:, 0] = slab[:, 1::2]        # d1 (odd x)
        zw3[:, 1] = zd[:, 0::2]          # d0-d1 (even x)
        zw3[:, 2] = zd[:, 1::2]          # d1-d2 (negated g1 pairs with it)
        zw3 = np.ascontiguousarray(zw3.astype(F8))

        # ztp slots: slot(k_rank, e, par) = D_par[32k + e + p] where
        # D_par[y] = z2T[2y + par]
        slabT = slab.T                                       # [1536, 128]
        ztp = np.zeros((128, NSLOT, KC), dtype=F8)
        for k in range(18):
            for e in range(3):
                for par in range(2):
                    slot = 6 * KRANK[k] + 2 * e + par
                    base = 64 * k + 2 * e + par
                    ztp[:, slot, :] = slabT[base : base + 256 : 2]

        # masks in parity-split order: ci = 2*blk + par,
        # row p -> position x = 2*(128*blk + p) + par
        msk = np.zeros((128, NB), dtype=np.float32)
        for blk in range(3):
            for par in range(2):
                x = 2 * (128 * blk + p) + par
                real = ((x % W) < WK) & ((h0 + x // W) < HK)
                msk[:, 2 * blk + par] = real
        in_maps.append(
            {
                "zw3": zw3,
                "qw": qw,
                "ztp": np.ascontiguousarray(ztp),
                "msk": msk,
            }
        )

    # colsum[(c,i,j)] = sum of kv rows over real patches, via integral image
    I = np.zeros((KC, H + 1, W + 1), dtype=np.float64)
    I[:, 1:, 1:] = z2a.astype(np.float64).cumsum(axis=1).cumsum(axis=2)
    colsum = np.zeros((KC, KH, KW), dtype=np.float64)
    for i in range(KH):
        for j in range(KW):
            colsum[:, i, j] = (
                I[:, i + HK, j + WK] - I[:, i, j + WK] - I[:, i + HK, j] + I[:, i, j]
            )
    return in_maps, colsum.reshape(KC, NIJ)


def kernel(z1_hat, z2):
    from concourse.bass_utils import run_bass_kernel_spmd

    in_maps, colsum = _host_prep(z1_hat, z2)
    if "nc" not in _CACHE:
        _CACHE["nc"] = _build_nc()
    nc = _CACHE["nc"]
    res = run_bass_kernel_spmd(nc, in_maps, list(range(NCORES)))
    num = colsum[:, :, None].astype(np.float64).copy()     # [128, 40, 1]
    num = np.broadcast_to(num, (KC, NIJ, PQ)).copy()
    den = np.full((PQ,), float(HK * WK), dtype=np.float64)
    for r in res.results:
        num += r["out"].astype(np.float64)
        den += r["den"].astype(np.float64)[0]
    out = num / den[None, None, :]
    # fold: [c, (i,j), q=(nh,nw)] -> [1, 128, 100, 64]
    arr = out.reshape(KC, KH, KW, NH, NW).transpose(0, 3, 1, 4, 2)
    return np.ascontiguousarray(arr.reshape(1, KC, H, W).astype(np.float32))
